# revision 23
# baseline (speedup 1.0000x reference)
"""nn_C3PartialConv — v7.6: row-tiled K=60 band pairs, no q-duplication.

Banded Toeplitz formulation with PE row-tiling instead of column-shift
(q) folding:
  - Band = 6 output rows (needs 10 input rows x 6 cin = K 60).
  - Two bands run CONCURRENTLY as row tiles of the 128x128 PE array:
    even band on partitions 0-59 (tile_position (0,0)), odd band on
    partitions 64-123 (tile_position (64,0)).  One XBUS column carries
    both streams on disjoint partition lanes; a pair of N=504 matmuls
    costs ~213ns (= N/2.4GHz) with self-loading LDWEIGHTS fully hidden
    by the alternating row groups.
  - The 5 kernel columns dj are 5 accumulating matmuls per band with a
    free-dim window shift (252 + 4 = 256, so no pad column needed).
  - M = 96 = 6 out rows x 16 cout; N = 504 = 2 images x 252 cols.
  - x HBM traffic halves vs the q-folded v6 (no duplicated shift copy);
    x loads are chunked (pair 0: 2/3/8/8 band pairs so the first matmul
    starts right after DMA-queue spin-up; later pairs 8/8/5 so the
    final og store and ACT are small => short tail).
  - tanh+bias applied by ScalarE over FOUR PSUM banks per instruction
    ([96, 2040], bands padded to 512 f32 = one bank each) to amortize
    the ~352-cycle ACT overhead; the 8 garbage columns between bands
    are dropped on the host.
  - ~3.4us of memset-sourced warm-up matmuls run during DMA-queue
    spin-up to lift the PE HAM clock gate before the real stream.
  - og stores are split per chunk and the final ones go via HWDGE.
"""

import os
import sys
import types

import numpy as np
import ml_dtypes

import concourse.bass as bass
import concourse.tile as tile
from concourse import mybir
from concourse.bass_utils import run_bass_kernel_spmd


def _ensure_ntff_hook():
    """bass_utils' BASS_TRACE path imports antenv.axon_hooks, which this
    image lacks; register a ctypes-backed stand-in so tracing works (or
    degrades gracefully) instead of crashing on import."""
    try:
        import antenv

        try:
            import antenv.axon_hooks  # noqa: F401

            return
        except ImportError:
            pass
        hook = None
        try:
            from trn_agent_boot.trn_boot import _ntff_profile_via_ctypes

            hook = _ntff_profile_via_ctypes("/opt/axon/libaxon_pjrt.so")
        except Exception:
            hook = None
        mod = types.ModuleType("antenv.axon_hooks")
        mod.get_axon_ntff_profile_hook = lambda: hook
        mod.set_axon_ntff_profile_hook = lambda h: None
        sys.modules["antenv.axon_hooks"] = mod
        antenv.axon_hooks = mod
    except Exception:
        pass


_ensure_ntff_hook()

C3_CONNECTIONS = [
    [0, 1, 2], [1, 2, 3], [2, 3, 4], [3, 4, 5], [4, 5, 0], [5, 0, 1],
    [0, 1, 2, 3], [1, 2, 3, 4], [2, 3, 4, 5], [3, 4, 5, 0], [4, 5, 0, 1],
    [5, 0, 1, 2], [0, 1, 3, 4], [1, 2, 4, 5], [0, 2, 3, 5],
    [0, 1, 2, 3, 4, 5],
]

B, CIN, H, W_IMG = 64, 6, 256, 256
COUT, KH, KW = 16, 5, 5
OH = OW = 252
N_CORES = 8
PER = B // N_CORES          # 8 images per core
NPAIR = PER // 2            # 4 image pairs per core
G, S = 6, 10                # out rows / in rows per band
K, M = S * CIN, G * COUT    # 60, 96
NFREE = 2 * OW              # 504
NB = 42                     # bands per image pair (42*6 = 252)
NBP = NB // 2               # 21 band pairs
BANDW = 2 * W_IMG           # 512 free elems per (s,c) band row

# x chunks per pair: (bp start, n bp).  Pair 0 leads with a tiny chunk
# so the first matmul starts right after DMA-queue spin-up; the last
# pair ends small so the final og store (and final ACT) are small.
CHUNKS_FIRST = [(0, 2), (2, 3), (5, 8), (13, 8)]
CHUNKS_REST = [(0, 8), (8, 8), (16, 5)]
# psum blocks per chunk size: (local bp start, n bp)
BLOCKS_BY_N = {
    2: [(0, 2)],
    3: [(0, 2), (2, 1)],
    5: [(0, 2), (2, 2), (4, 1)],
    8: [(0, 2), (2, 2), (4, 2), (6, 2)],
}
OG_BY_N = {2: 2040, 3: 3056, 5: 5096, 8: 8160}
OG_PAIR = 21416             # og cols per pair

BF = mybir.dt.bfloat16
F32 = mybir.dt.float32
NP_BF = ml_dtypes.bfloat16

SCALE_IN = 2.0 / 3.0
SCALE_OUT = 1.7159


def _mask() -> np.ndarray:
    m = np.zeros((COUT, CIN, KH, KW), dtype=np.float32)
    for i, conn in enumerate(C3_CONNECTIONS):
        m[i, conn] = 1.0
    return m


def _pack_weights(Wm: np.ndarray) -> np.ndarray:
    """[16,6,5,5] -> [128, 5*96].
    row p = s*6 + c (and 64+p duplicate); col = dj*96 + i*16 + o;
    value Wm[o, c, s-i, dj] for 0 <= s-i < 5."""
    wp = np.zeros((64, KW, G, COUT), dtype=np.float32)
    for i in range(G):
        for di in range(KH):
            s = i + di
            for c in range(CIN):
                wp[s * CIN + c, :, i, :] = Wm[:, c, di, :].T
    out = np.zeros((128, KW * M), dtype=np.float32)
    out[0:64] = wp.reshape(64, KW * M)
    out[64:128] = out[0:64]
    return out


def _pack_bias(b: np.ndarray) -> np.ndarray:
    """[16] -> [M, 1]: (2/3)*b[o] at partition i*16+o."""
    bm = np.empty((G, COUT), dtype=np.float32)
    bm[:] = SCALE_IN * b[None, :]
    return bm.reshape(M, 1)


def _pack_x(xs_core: np.ndarray, dtype) -> np.ndarray:
    """[PER,6,256,256] -> [NPAIR, 120, NBP*512].

    Row p = s*6+c holds even-band rows (image row 12j+s), row 60+p holds
    odd-band rows (image row 12j+6+s); free = j*512 + m*256 + w.  On
    device the halves land on SBUF partitions 0-59 / 64-123.
    """
    xp = xs_core.reshape(NPAIR, 2, CIN, H, W_IMG)
    st = xp.strides
    # full[n, m, c, j, t, w] = xp[n, m, c, 12j + t, w],  t in 0..15
    full = np.lib.stride_tricks.as_strided(
        xp,
        shape=(NPAIR, 2, CIN, NBP, 16, W_IMG),
        strides=(st[0], st[1], st[2], 12 * st[3], st[3], st[4]),
    )
    out = np.zeros((NPAIR, 128, NBP, 2, W_IMG), dtype=dtype)
    # -> [n, s, c, j, m, w]
    out[:, 0:K] = full[:, :, :, :, 0:S].transpose(0, 4, 2, 3, 1, 5).reshape(
        NPAIR, K, NBP, 2, W_IMG)
    out[:, 64:64 + K] = full[:, :, :, :, 6:6 + S].transpose(
        0, 4, 2, 3, 1, 5).reshape(NPAIR, K, NBP, 2, W_IMG)
    return out.reshape(NPAIR, 128, NBP * BANDW)


# og column segments per pair: (col, n bands); bands are 504 valid cols
# at 512-col stride within a segment; segments cover bands in order.
def _og_segments(chunks):
    segs = []
    col = 0
    for _, nbp in chunks:
        for _, bsz in BLOCKS_BY_N[nbp]:
            segs.append((col, 2 * bsz))
            col += 2040 if bsz == 2 else 1016
    assert col == OG_PAIR
    return segs


OG_SEGS_FIRST = _og_segments(CHUNKS_FIRST)
OG_SEGS_REST = _og_segments(CHUNKS_REST)


def _unpack_y_into(y_dev: np.ndarray, out: np.ndarray) -> None:
    """[NPAIR, 96, OG_PAIR] -> out [PER,16,252,252] (unscaled)."""
    bands = np.empty((NPAIR, M, NB, NFREE), dtype=y_dev.dtype)
    st = y_dev.strides
    for n in range(NPAIR):
        segs = OG_SEGS_FIRST if n == 0 else OG_SEGS_REST
        b = 0
        for col, nb in segs:
            seg = np.lib.stride_tricks.as_strided(
                y_dev[n, :, col:],
                shape=(M, nb, NFREE),
                strides=(st[1], 512 * st[2], st[2]),
            )
            bands[n, :, b:b + nb] = seg
            b += nb
    # [n, i, o, b, m, w] -> [n, m, o, (b, i), w]
    yd = bands.reshape(NPAIR, G, COUT, NB, 2, OW).transpose(0, 4, 2, 3, 1, 5)
    out.reshape(NPAIR, 2, COUT, NB * G, OW)[...] = yd.reshape(
        NPAIR, 2, COUT, NB * G, OW)


def _split_excess_syncs(nc):
    def budget(ins):
        return 1 if isinstance(ins, (mybir.InstDrain, mybir.InstNoOp)) else 2

    for bb in nc.m.functions[0].blocks:
        new_insts = []
        for ins in bb.instructions:
            si = ins.sync_info
            w = list(si.on_wait) if si and si.on_wait else []
            u = list(si.on_update) if si and si.on_update else []
            cap = budget(ins)
            if len(w) + len(u) > cap:
                keep_n = max(0, cap - len(u))
                excess, kept = w[: len(w) - keep_n], w[len(w) - keep_n:]
                for wait in excess:
                    new_insts.append(
                        mybir.InstNoOp(
                            name=nc.get_next_instruction_name(),
                            sync_info=mybir.SyncInfo(on_wait=[wait],
                                                     on_update=[]),
                            bass_nofuse=True,
                            engine=ins.engine,
                        )
                    )
                ins.sync_info = mybir.SyncInfo(on_wait=kept, on_update=u)
            new_insts.append(ins)
        bb.instructions[:] = new_insts


def _build_nc(iters: int = 1):
    nc = bass.Bass()
    x = nc.declare_dram_parameter("x", [NPAIR, 128, NBP * BANDW], BF,
                                  isOutput=False)
    wm = nc.declare_dram_parameter("wm", [128, KW * M], BF, isOutput=False)
    bm = nc.declare_dram_parameter("bm", [M, 1], F32, isOutput=False)
    y = nc.declare_dram_parameter("y", [NPAIR, M, OG_PAIR], BF,
                                  isOutput=True)

    with tile.TileContext(nc) as tc:
        with (
            tc.tile_pool(name="consts", bufs=1) as consts,
            tc.tile_pool(name="x2", bufs=1) as x2pool,
            tc.tile_pool(name="x3", bufs=1) as x3pool,
            tc.tile_pool(name="x5", bufs=2) as x5pool,
            tc.tile_pool(name="x8", bufs=4) as x8pool,
            tc.tile_pool(name="ps", bufs=2, space="PSUM") as pspool,
            tc.tile_pool(name="o2", bufs=1) as o2pool,
            tc.tile_pool(name="o3", bufs=1) as o3pool,
            tc.tile_pool(name="o5", bufs=2) as o5pool,
            tc.tile_pool(name="o8", bufs=3) as o8pool,
        ):
            xpools = {2: x2pool, 3: x3pool, 5: x5pool, 8: x8pool}
            opools = {2: o2pool, 3: o3pool, 5: o5pool, 8: o8pool}
            wt = consts.tile([128, KW * M], BF)
            nc.sync.dma_start(out=wt[:, :], in_=wm[:, :])
            bt = consts.tile([M, 1], F32)
            nc.sync.dma_start(out=bt[:, :], in_=bm[:, :])
            # preload the tanh table set
            warm = consts.tile([1, 1], F32)
            nc.scalar.activation(out=warm[:, :], in_=bt[0:1, :],
                                 func=mybir.ActivationFunctionType.Tanh)
            # HAM warm-up: ~3.4us of tiny matmuls during DMA-queue spin-up;
            # memset-sourced so they need no DMA at all.
            wmt = consts.tile([K, 128], BF)
            nc.gpsimd.memset(wmt[:, :], 0.25)
            wps = pspool.tile([M, 2048], F32, tag="ps", name="wps")
            for _ in range(32):
                nc.tensor.matmul(wps[0:16, 0:128], wmt[:, 0:16],
                                 wmt[:, :], start=True, stop=True)

            def body(_iv=None):
                for pair in range(NPAIR):
                    chunks = CHUNKS_FIRST if pair == 0 else CHUNKS_REST
                    ogcol = 0
                    for ci, (bp0, nbp) in enumerate(chunks):
                        xt = xpools[nbp].tile(
                            [128, nbp * BANDW], BF,
                            tag=f"x{nbp}", name="xt")
                        c0, c1 = bp0 * BANDW, (bp0 + nbp) * BANDW
                        nc.sync.dma_start(out=xt[:, :],
                                          in_=x[pair, :, c0:c1])
                        ogw = OG_BY_N[nbp]
                        og = opools[nbp].tile(
                            [M, ogw], BF,
                            tag=f"o{nbp}", name="og")
                        goff = 0
                        for lb0, bsz in BLOCKS_BY_N[nbp]:
                            tc.no_sync_barrier()
                            ps = pspool.tile([M, 2048], F32, tag="ps",
                                             name="ps")
                            for dj in range(KW):
                                for u in range(bsz):
                                    lv = lb0 + u
                                    xv = xt[:, lv * BANDW:
                                            (lv + 1) * BANDW].rearrange(
                                        "k (m w) -> k m w", m=2)
                                    for h in range(2):
                                        off = u * 1024 + h * 512
                                        nc.tensor.matmul(
                                            ps[:, off:off + NFREE],
                                            wt[h * 64:h * 64 + K,
                                               dj * M:(dj + 1) * M],
                                            xv[h * 64:h * 64 + K, :,
                                               dj:dj + OW],
                                            start=(dj == 0),
                                            stop=(dj == KW - 1),
                                        )
                            width = 2040 if bsz == 2 else 1016
                            nc.scalar.activation(
                                out=og[:, goff:goff + width],
                                in_=ps[:, 0:width],
                                func=mybir.ActivationFunctionType.Tanh,
                                bias=bt[:, 0:1],
                                scale=SCALE_IN,
                            )
                            goff += width
                        # split store: overlap output DMA, shrink tail
                        last = pair == NPAIR - 1 and ci == len(chunks) - 1
                        if last:
                            cuts = [0, 2040, 4080, ogw]
                        elif ogw > 4080:
                            cuts = [0, 4080, ogw]
                        else:
                            cuts = [0, ogw]
                        # final stores go HWDGE (sync): lower completion
                        # latency, and the x-load ring is drained by then
                        eng = nc.sync if last else nc.gpsimd
                        for a, bnd in zip(cuts[:-1], cuts[1:]):
                            eng.dma_start(
                                out=y[pair][:, ogcol + a:ogcol + bnd],
                                in_=og[:, a:bnd])
                        ogcol += ogw

            for _ in range(iters):
                body()
    _split_excess_syncs(nc)
    return nc


_NC_CACHE = {}
LAST_EXEC_NS = None


def kernel(x: np.ndarray, W: np.ndarray, b: np.ndarray) -> np.ndarray:
    global LAST_EXEC_NS
    x = np.asarray(x, dtype=np.float32)
    W = np.asarray(W, dtype=np.float32)
    b = np.asarray(b, dtype=np.float32)

    wp = _pack_weights(W * _mask()).astype(NP_BF)
    bm = _pack_bias(b)
    xs = x.reshape(N_CORES, PER, CIN, H, W_IMG)

    iters = int(os.environ.get("KERNEL_ITERS", "1"))
    if iters not in _NC_CACHE:
        _NC_CACHE[iters] = _build_nc(iters)
    nc = _NC_CACHE[iters]

    in_maps = [
        {"x": _pack_x(xs[i], dtype=NP_BF), "wm": wp, "bm": bm}
        for i in range(N_CORES)
    ]
    res = run_bass_kernel_spmd(nc, in_maps, list(range(N_CORES)))
    LAST_EXEC_NS = res.exec_time_ns
    y = np.empty((B, COUT, OH, OW), dtype=np.float32)
    for i in range(N_CORES):
        _unpack_y_into(
            np.asarray(res.results[i]["y"], dtype=np.float32),
            y[i * PER:(i + 1) * PER],
        )
    y *= np.float32(SCALE_OUT)
    return y


# revision 28
# speedup vs baseline: 1.0250x; 1.0250x over previous
"""nn_C3PartialConv — v7.6: row-tiled K=60 band pairs, no q-duplication.

Banded Toeplitz formulation with PE row-tiling instead of column-shift
(q) folding:
  - Band = 6 output rows (needs 10 input rows x 6 cin = K 60).
  - Two bands run CONCURRENTLY as row tiles of the 128x128 PE array:
    even band on partitions 0-59 (tile_position (0,0)), odd band on
    partitions 64-123 (tile_position (64,0)).  One XBUS column carries
    both streams on disjoint partition lanes; a pair of N=504 matmuls
    costs ~213ns (= N/2.4GHz) with self-loading LDWEIGHTS fully hidden
    by the alternating row groups.
  - The 5 kernel columns dj are 5 accumulating matmuls per band with a
    free-dim window shift (252 + 4 = 256, so no pad column needed).
  - M = 96 = 6 out rows x 16 cout; N = 504 = 2 images x 252 cols.
  - x HBM traffic halves vs the q-folded v6 (no duplicated shift copy);
    x loads are chunked (pair 0: 2/3/8/8 band pairs so the first matmul
    starts right after DMA-queue spin-up; later pairs 8/8/5 so the
    final og store and ACT are small => short tail).
  - tanh+bias applied by ScalarE over FOUR PSUM banks per instruction
    ([96, 2040], bands padded to 512 f32 = one bank each) to amortize
    the ~352-cycle ACT overhead; the 8 garbage columns between bands
    are dropped on the host.
  - ~3.4us of memset-sourced warm-up matmuls run during DMA-queue
    spin-up to lift the PE HAM clock gate before the real stream.
  - og stores are split per chunk and the final ones go via HWDGE.
"""

import os
import sys
import types

import numpy as np
import ml_dtypes

import concourse.bass as bass
import concourse.tile as tile
from concourse import mybir
from concourse.bass_utils import run_bass_kernel_spmd


def _ensure_ntff_hook():
    """bass_utils' BASS_TRACE path imports antenv.axon_hooks, which this
    image lacks; register a ctypes-backed stand-in so tracing works (or
    degrades gracefully) instead of crashing on import."""
    try:
        import antenv

        try:
            import antenv.axon_hooks  # noqa: F401

            return
        except ImportError:
            pass
        hook = None
        try:
            from trn_agent_boot.trn_boot import _ntff_profile_via_ctypes

            hook = _ntff_profile_via_ctypes("/opt/axon/libaxon_pjrt.so")
        except Exception:
            hook = None
        mod = types.ModuleType("antenv.axon_hooks")
        mod.get_axon_ntff_profile_hook = lambda: hook
        mod.set_axon_ntff_profile_hook = lambda h: None
        sys.modules["antenv.axon_hooks"] = mod
        antenv.axon_hooks = mod
    except Exception:
        pass


_ensure_ntff_hook()

C3_CONNECTIONS = [
    [0, 1, 2], [1, 2, 3], [2, 3, 4], [3, 4, 5], [4, 5, 0], [5, 0, 1],
    [0, 1, 2, 3], [1, 2, 3, 4], [2, 3, 4, 5], [3, 4, 5, 0], [4, 5, 0, 1],
    [5, 0, 1, 2], [0, 1, 3, 4], [1, 2, 4, 5], [0, 2, 3, 5],
    [0, 1, 2, 3, 4, 5],
]

B, CIN, H, W_IMG = 64, 6, 256, 256
COUT, KH, KW = 16, 5, 5
OH = OW = 252
N_CORES = 8
PER = B // N_CORES          # 8 images per core
NPAIR = PER // 2            # 4 image pairs per core
G, S = 6, 10                # out rows / in rows per band
K, M = S * CIN, G * COUT    # 60, 96
NFREE = 2 * OW              # 504
NB = 42                     # bands per image pair (42*6 = 252)
NBP = NB // 2               # 21 band pairs
BANDW = 2 * W_IMG           # 512 free elems per (s,c) band row

# x chunks per pair: (bp start, n bp).  Pair 0 leads with a tiny chunk
# so the first matmul starts right after DMA-queue spin-up.  PSUM blocks
# are decoupled from x chunks: every block is exactly 2 band pairs (4
# bands, 4 PSUM banks); since 21 bp/pair is odd, two blocks CROSS pair
# boundaries (pair0 bp20 + pair1 bp0, pair2 bp20 + pair3 bp0) so no
# undersized block ever stalls the PE on PSUM recycling.
CHUNKS_FIRST = [(0, 2), (2, 3), (5, 8), (13, 8)]
CHUNKS_REST = [(0, 8), (8, 8), (16, 5)]
OG_PAIR = 21416             # og cols per pair
# own-og tiles per pair: (n blocks, width, y col); even pairs also own
# the cross tile holding their bands 40-41 (y col 20400) plus the next
# pair's bands 0-1 (its y col 0).
OG_PLAN_EVEN = [(4, 8160, 0), (4, 8160, 8160), (2, 4080, 16320)]
OG_PLAN_ODD = [(4, 8160, 1016), (4, 8160, 9176), (2, 4080, 17336)]
OG_SEGS_EVEN = [(2040 * i, 4) for i in range(10)] + [(20400, 2)]
OG_SEGS_ODD = [(0, 2)] + [(1016 + 2040 * i, 4) for i in range(10)]

BF = mybir.dt.bfloat16
F32 = mybir.dt.float32
NP_BF = ml_dtypes.bfloat16

SCALE_IN = 2.0 / 3.0
SCALE_OUT = 1.7159


def _mask() -> np.ndarray:
    m = np.zeros((COUT, CIN, KH, KW), dtype=np.float32)
    for i, conn in enumerate(C3_CONNECTIONS):
        m[i, conn] = 1.0
    return m


def _pack_weights(Wm: np.ndarray) -> np.ndarray:
    """[16,6,5,5] -> [128, 5*96].
    row p = s*6 + c (and 64+p duplicate); col = dj*96 + i*16 + o;
    value Wm[o, c, s-i, dj] for 0 <= s-i < 5."""
    wp = np.zeros((64, KW, G, COUT), dtype=np.float32)
    for i in range(G):
        for di in range(KH):
            s = i + di
            for c in range(CIN):
                wp[s * CIN + c, :, i, :] = Wm[:, c, di, :].T
    out = np.zeros((128, KW * M), dtype=np.float32)
    out[0:64] = wp.reshape(64, KW * M)
    out[64:128] = out[0:64]
    return out


def _pack_bias(b: np.ndarray) -> np.ndarray:
    """[16] -> [M, 1]: (2/3)*b[o] at partition i*16+o."""
    bm = np.empty((G, COUT), dtype=np.float32)
    bm[:] = SCALE_IN * b[None, :]
    return bm.reshape(M, 1)


def _pack_x(xs_core: np.ndarray, dtype) -> np.ndarray:
    """[PER,6,256,256] -> [NPAIR, 128, NBP*512].

    Partition p = s*6+c holds even-band rows (image row 12j+s), partition
    64+p holds odd-band rows (image row 12j+6+s); free = j*512 + m*256 + w.
    Partitions 60-63 / 124-127 are zero padding (kept for a single
    balanced 128-partition DMA per chunk).
    """
    xp = xs_core.reshape(NPAIR, 2, CIN, H, W_IMG)
    st = xp.strides
    # full[n, m, c, j, t, w] = xp[n, m, c, 12j + t, w],  t in 0..15
    full = np.lib.stride_tricks.as_strided(
        xp,
        shape=(NPAIR, 2, CIN, NBP, 16, W_IMG),
        strides=(st[0], st[1], st[2], 12 * st[3], st[3], st[4]),
    )
    out = np.zeros((NPAIR, 128, NBP, 2, W_IMG), dtype=dtype)
    # -> [n, s, c, j, m, w]
    out[:, 0:K] = full[:, :, :, :, 0:S].transpose(0, 4, 2, 3, 1, 5).reshape(
        NPAIR, K, NBP, 2, W_IMG)
    out[:, 64:64 + K] = full[:, :, :, :, 6:6 + S].transpose(
        0, 4, 2, 3, 1, 5).reshape(NPAIR, K, NBP, 2, W_IMG)
    return out.reshape(NPAIR, 128, NBP * BANDW)


def _unpack_y_into(y_dev: np.ndarray, out: np.ndarray) -> None:
    """[NPAIR, 96, OG_PAIR] -> out [PER,16,252,252] (unscaled).

    og column segments per pair: (col, n bands); bands are 504 valid
    cols at 512-col stride within a segment, in band order."""
    bands = np.empty((NPAIR, M, NB, NFREE), dtype=y_dev.dtype)
    st = y_dev.strides
    for n in range(NPAIR):
        segs = OG_SEGS_EVEN if n % 2 == 0 else OG_SEGS_ODD
        b = 0
        for col, nb in segs:
            seg = np.lib.stride_tricks.as_strided(
                y_dev[n, :, col:],
                shape=(M, nb, NFREE),
                strides=(st[1], 512 * st[2], st[2]),
            )
            bands[n, :, b:b + nb] = seg
            b += nb
    # [n, i, o, b, m, w] -> [n, m, o, (b, i), w]
    yd = bands.reshape(NPAIR, G, COUT, NB, 2, OW).transpose(0, 4, 2, 3, 1, 5)
    out.reshape(NPAIR, 2, COUT, NB * G, OW)[...] = yd.reshape(
        NPAIR, 2, COUT, NB * G, OW)


def _split_excess_syncs(nc):
    def budget(ins):
        return 1 if isinstance(ins, (mybir.InstDrain, mybir.InstNoOp)) else 2

    for bb in nc.m.functions[0].blocks:
        new_insts = []
        for ins in bb.instructions:
            si = ins.sync_info
            w = list(si.on_wait) if si and si.on_wait else []
            u = list(si.on_update) if si and si.on_update else []
            cap = budget(ins)
            if len(w) + len(u) > cap:
                keep_n = max(0, cap - len(u))
                excess, kept = w[: len(w) - keep_n], w[len(w) - keep_n:]
                for wait in excess:
                    new_insts.append(
                        mybir.InstNoOp(
                            name=nc.get_next_instruction_name(),
                            sync_info=mybir.SyncInfo(on_wait=[wait],
                                                     on_update=[]),
                            bass_nofuse=True,
                            engine=ins.engine,
                        )
                    )
                ins.sync_info = mybir.SyncInfo(on_wait=kept, on_update=u)
            new_insts.append(ins)
        bb.instructions[:] = new_insts


def _build_nc(iters: int = 1):
    nc = bass.Bass()
    x = nc.declare_dram_parameter("x", [NPAIR, 128, NBP * BANDW], BF,
                                  isOutput=False)
    wm = nc.declare_dram_parameter("wm", [128, KW * M], BF, isOutput=False)
    bm = nc.declare_dram_parameter("bm", [M, 1], F32, isOutput=False)
    y = nc.declare_dram_parameter("y", [NPAIR, M, OG_PAIR], BF,
                                  isOutput=True)

    with tile.TileContext(nc) as tc:
        with (
            tc.tile_pool(name="consts", bufs=1) as consts,
            tc.tile_pool(name="x2", bufs=1) as x2pool,
            tc.tile_pool(name="x3", bufs=1) as x3pool,
            tc.tile_pool(name="x5", bufs=2) as x5pool,
            tc.tile_pool(name="x8", bufs=4) as x8pool,
            tc.tile_pool(name="ps", bufs=2, space="PSUM") as pspool,
            tc.tile_pool(name="oc", bufs=2) as ocpool,
            tc.tile_pool(name="o4", bufs=2) as o4pool,
            tc.tile_pool(name="o8", bufs=3) as o8pool,
        ):
            xpools = {2: x2pool, 3: x3pool, 5: x5pool, 8: x8pool}
            opools = {2040: ocpool, 4080: o4pool, 8160: o8pool}
            wt = consts.tile([128, KW * M], BF)
            nc.sync.dma_start(out=wt[:, :], in_=wm[:, :])
            bt = consts.tile([M, 1], F32)
            nc.sync.dma_start(out=bt[:, :], in_=bm[:, :])
            # preload the tanh table set
            warm = consts.tile([1, 1], F32)
            nc.scalar.activation(out=warm[:, :], in_=bt[0:1, :],
                                 func=mybir.ActivationFunctionType.Tanh)
            # HAM warm-up: ~3.4us of tiny matmuls during DMA-queue spin-up;
            # memset-sourced so they need no DMA at all.
            wmt = consts.tile([K, 128], BF)
            nc.gpsimd.memset(wmt[:, :], 0.25)
            wps = pspool.tile([M, 2048], F32, tag="ps", name="wps")
            for _ in range(32):
                nc.tensor.matmul(wps[0:16, 0:128], wmt[:, 0:16],
                                 wmt[:, :], start=True, stop=True)

            def body(_iv=None):
                xts = {}

                def get_xt(p, bp):
                    """Chunk tile holding band pair (p, bp); issues the
                    load at first touch (prefetch depth = pool bufs)."""
                    chunks = CHUNKS_FIRST if p == 0 else CHUNKS_REST
                    for ci, (b0, nb) in enumerate(chunks):
                        if b0 <= bp < b0 + nb:
                            key = (p, ci)
                            if key not in xts:
                                xt = xpools[nb].tile(
                                    [128, nb * BANDW], BF,
                                    tag=f"x{nb}", name="xt")
                                nc.sync.dma_start(
                                    out=xt[:, :],
                                    in_=x[p, :,
                                          b0 * BANDW:(b0 + nb) * BANDW])
                                xts[key] = (xt, b0)
                            return xts[key]
                    raise AssertionError((p, bp))

                def emit_block(bands2, og, goff):
                    """One PSUM block: 2 band pairs (possibly from two
                    different image pairs / x chunks), 4 bands."""
                    for p, bp in bands2:
                        get_xt(p, bp)
                    tc.no_sync_barrier()
                    ps = pspool.tile([M, 2048], F32, tag="ps", name="ps")
                    for dj in range(KW):
                        for u, (p, bp) in enumerate(bands2):
                            xt, b0 = get_xt(p, bp)
                            lv = bp - b0
                            xv = xt[:, lv * BANDW:
                                    (lv + 1) * BANDW].rearrange(
                                "k (m w) -> k m w", m=2)
                            for h in range(2):
                                off = u * 1024 + h * 512
                                nc.tensor.matmul(
                                    ps[:, off:off + NFREE],
                                    wt[h * 64:h * 64 + K,
                                       dj * M:(dj + 1) * M],
                                    xv[h * 64:h * 64 + K, :, dj:dj + OW],
                                    start=(dj == 0),
                                    stop=(dj == KW - 1),
                                )
                    nc.scalar.activation(
                        out=og[:, goff:goff + 2040],
                        in_=ps[:, 0:2040],
                        func=mybir.ActivationFunctionType.Tanh,
                        bias=bt[:, 0:1],
                        scale=SCALE_IN,
                    )

                for pair in range(NPAIR):
                    own = ([(b, b + 1) for b in range(0, 20, 2)]
                           if pair % 2 == 0 else
                           [(b, b + 1) for b in range(1, 21, 2)])
                    plan = OG_PLAN_EVEN if pair % 2 == 0 else OG_PLAN_ODD
                    bi = 0
                    for ti, (nblk, width, ycol) in enumerate(plan):
                        og = opools[width].tile([M, width], BF,
                                                tag=f"o{width}", name="og")
                        for k in range(nblk):
                            bps = own[bi]
                            bi += 1
                            emit_block([(pair, bps[0]), (pair, bps[1])],
                                       og, k * 2040)
                        # split store: overlap output DMA, shrink tail
                        last = (pair == NPAIR - 1
                                and ti == len(plan) - 1)
                        if last:
                            cuts = [0, 2040, 4080]
                        elif width > 4080:
                            cuts = [0, 4080, 8160]
                        else:
                            cuts = [0, 4080]
                        # final stores go HWDGE (sync): lower completion
                        # latency, and the x-load ring is drained by then
                        eng = nc.sync if last else nc.gpsimd
                        for a, bnd in zip(cuts[:-1], cuts[1:]):
                            eng.dma_start(
                                out=y[pair][:, ycol + a:ycol + bnd],
                                in_=og[:, a:bnd])
                    if pair % 2 == 0:
                        # cross block: this pair's bands 40-41 + the next
                        # pair's bands 0-1 in one full-size PSUM block
                        ogx = ocpool.tile([M, 2040], BF, tag="o2040",
                                          name="ogx")
                        emit_block([(pair, 20), (pair + 1, 0)], ogx, 0)
                        nc.gpsimd.dma_start(out=y[pair][:, 20400:21416],
                                            in_=ogx[:, 0:1016])
                        nc.gpsimd.dma_start(out=y[pair + 1][:, 0:1016],
                                            in_=ogx[:, 1024:2040])

            for _ in range(iters):
                body()
    _split_excess_syncs(nc)
    return nc


_NC_CACHE = {}
LAST_EXEC_NS = None


def kernel(x: np.ndarray, W: np.ndarray, b: np.ndarray) -> np.ndarray:
    global LAST_EXEC_NS
    x = np.asarray(x, dtype=np.float32)
    W = np.asarray(W, dtype=np.float32)
    b = np.asarray(b, dtype=np.float32)

    wp = _pack_weights(W * _mask()).astype(NP_BF)
    bm = _pack_bias(b)
    xs = x.reshape(N_CORES, PER, CIN, H, W_IMG)

    iters = int(os.environ.get("KERNEL_ITERS", "1"))
    if iters not in _NC_CACHE:
        _NC_CACHE[iters] = _build_nc(iters)
    nc = _NC_CACHE[iters]

    in_maps = [
        {"x": _pack_x(xs[i], dtype=NP_BF), "wm": wp, "bm": bm}
        for i in range(N_CORES)
    ]
    res = run_bass_kernel_spmd(nc, in_maps, list(range(N_CORES)))
    LAST_EXEC_NS = res.exec_time_ns
    y = np.empty((B, COUT, OH, OW), dtype=np.float32)
    for i in range(N_CORES):
        _unpack_y_into(
            np.asarray(res.results[i]["y"], dtype=np.float32),
            y[i * PER:(i + 1) * PER],
        )
    y *= np.float32(SCALE_OUT)
    return y
